# revision 1
# baseline (speedup 1.0000x reference)
"""3-layer GAT (8 heads x 32 hid, PyG GATConv semantics w/ self-loops) +
global mean pool + linear, distributed over 8 Trainium2 NeuronCores.

Strategy (per sharding hint): nodes partitioned into 8 contiguous ranges
(dst-owner); incident edges live with their dst core, sorted by dst then src.
Each layer: node phase computes hh=[h|alpha_src] and alpha_dst for local
nodes, AllGather replicates hh; edge phase gathers hh[src] rows (indirect
DMA, 128 rows/instr), computes un-normalized attention numerator and
denominator with one-hot segment matmuls accumulated in PSUM, then divides
per dst node (segment softmax is algebraically folded: out = sum(ex*h)/sum(ex),
no max subtraction needed since e is O(1)-bounded). Per-graph mean-pool
partials are scattered into a 512-row buffer and AllReduced.

Self-contained: hardcodes the problem shapes; host-side preprocessing uses
only graph structure (edge_index, batch) and parameter repacking.
"""
import math
import numpy as np

import concourse.bass as bass
import concourse.bacc as bacc
import concourse.mybir as mybir
import concourse.tile as tile

P = 128
KB = 4
SKIP_NODE = False
SKIP_EDGE = False
SKIP_AG = False
E_GATHER = True
E_ATT = True
E_MUL = True
E_SMM = True
E_EPI = True
WORK_BUFS = 6
GATHER_BUFS = 4
PSA_BUFS = 2
PSB_BUFS = 2
HEADS, HID = 8, 32
DH = HEADS * HID          # 256
DA = DH + HEADS           # 264 = h | alpha_src
DW = DH + 2 * HEADS       # 272 = W | W@Asrc | W@Adst
DG = 320                  # gathered row: DA padded so DG*4B % 256B == 0
IN_CH = 128
NEG = 0.2
F32 = mybir.dt.float32
I32 = mybir.dt.int32


# ----------------------------------------------------------------- host prep

def host_prep(x, edge_index, batch, Ws, a_srcs, a_dsts, biases, lin_w, lin_b,
              N, E, G, ncores):
    """Build per-core input maps + the (core-uniform) program config.

    Edge layout: per dst tile, edges are split by src half-table (int16
    index limit of dma_gather), each half padded to whole 128-edge tiles;
    tiles are gathered in groups of <=KB via one dma_gather each.
    """
    nl_real = N // ncores
    assert nl_real * ncores == N
    NL = ((nl_real + P - 1) // P) * P          # padded local nodes
    NT = NL // P                               # dst tiles per core
    GP = ((G + P - 1) // P) * P                # padded graphs
    NG = NL * ncores
    HALF = NG // 2
    assert HALF <= 32768

    src = np.concatenate([edge_index[0], np.arange(N, dtype=np.int64)])
    dst = np.concatenate([edge_index[1], np.arange(N, dtype=np.int64)])
    core_of = dst // nl_real
    dloc_all = dst - core_of * nl_real

    # per-core, per-tile, per-half edge lists sorted by (dst_local, src)
    per_core = []
    clo = np.zeros((ncores, NT), np.int64)
    chi = np.zeros((ncores, NT), np.int64)
    for k in range(ncores):
        m = core_of == k
        s_k, d_k = src[m], dloc_all[m]
        gsrc = (s_k // nl_real) * NL + (s_k % nl_real)   # padded-global row
        order = np.lexsort((gsrc, d_k))
        gsrc, d_k = gsrc[order], d_k[order]
        t_k = d_k // P
        lo = gsrc < HALF
        tiles = []
        for t in range(NT):
            mt = t_k == t
            g_t, d_t, lo_t = gsrc[mt], d_k[mt] - t * P, lo[mt]
            glo, dlo = g_t[lo_t], d_t[lo_t]
            ghi, dhi = g_t[~lo_t] - HALF, d_t[~lo_t]
            if t == NT - 1 and NL > nl_real:
                npad = NL - nl_real
                glo = np.concatenate([glo, np.zeros(npad, np.int64)])
                dlo = np.concatenate([dlo, np.arange(nl_real - t * P,
                                                     nl_real - t * P + npad)])
            tiles.append(((glo, dlo), (ghi, dhi)))
            clo[k, t] = len(glo)
            chi[k, t] = len(ghi)
        per_core.append(tiles)
    m_lo = [int(v) for v in np.maximum(np.ceil(clo.max(axis=0) / P), 1).astype(np.int64)]
    m_hi = [int(v) for v in np.ceil(chi.max(axis=0) / P).astype(np.int64)]
    SM = int(sum(m_lo) + sum(m_hi))
    # column offset of each tile's run (lo tiles then hi tiles)
    col0 = []
    c = 0
    for t in range(NT):
        col0.append(c)
        c += m_lo[t] + m_hi[t]

    in_maps = []
    batch = np.asarray(batch, np.int64)
    for k in range(ncores):
        dstflat = np.full((P, SM), 999.0, np.float32)
        idx16 = np.zeros((16, 8 * SM), np.int16)
        for t in range(NT):
            for half, mh in ((0, m_lo[t]), (1, m_hi[t])):
                if mh == 0:
                    continue
                g_t, d_t = per_core[k][t][half]
                L = mh * P
                gi = np.zeros(L, np.int64)
                gi[:len(g_t)] = g_t
                dd = np.full(L, 999.0, np.float32)
                dd[:len(d_t)] = d_t
                c0 = col0[t] + (m_lo[t] if half else 0)
                ii = np.arange(L)
                dstflat[ii % P, c0 + ii // P] = dd
                # groups of <=KB tiles; idx wrap is per group
                j = 0
                while j < mh:
                    kb = min(KB, mh - j)
                    arr = gi[j * P:(j + kb) * P]
                    idx16[:, (c0 + j) * 8:(c0 + j + kb) * 8] = \
                        arr.reshape(-1, 16).T.astype(np.int16)
                    j += kb

        bl = batch[k * nl_real:(k + 1) * nl_real]
        g_base = int(bl[0])
        bshift = np.full(NL, 999.0, np.float32)
        bshift[:nl_real] = (bl - g_base).astype(np.float32)
        gslot = np.arange(g_base, g_base + P, dtype=np.int64)
        gslot = np.where(gslot < G, gslot, 99999).astype(np.int32)[:, None]

        xk = np.zeros((NL, IN_CH), np.float32)
        xk[:nl_real] = x[k * nl_real:(k + 1) * nl_real]

        im = {
            "x_local": xk,
            "idx16": np.tile(idx16, (8, 1)),
            "dstloc": dstflat.reshape(-1),
            "bshift": bshift,
            "gslot": gslot,
        }
        in_maps.append(im)

    # replicated parameters
    def aug(W, a_s, a_d):
        As = np.zeros((DH, HEADS), np.float32)
        Ad = np.zeros((DH, HEADS), np.float32)
        for h in range(HEADS):
            As[h * HID:(h + 1) * HID, h] = a_s[h]
            Ad[h * HID:(h + 1) * HID, h] = a_d[h]
        return np.concatenate([W, W @ As, W @ Ad], axis=1).astype(np.float32)

    cnts = np.bincount(batch, minlength=G).astype(np.float32)
    invcnt = np.zeros((GP, 1), np.float32)
    invcnt[:G, 0] = 1.0 / np.maximum(cnts, 1.0)
    params = {
        "W0aug": aug(Ws[0], a_srcs[0], a_dsts[0]),
        "W1aug": aug(Ws[1], a_srcs[1], a_dsts[1]),
        "W2aug": aug(Ws[2], a_srcs[2], a_dsts[2]),
        "b0": np.tile(biases[0][None, :], (P, 1)).astype(np.float32),
        "b1": np.tile(biases[1][None, :], (P, 1)).astype(np.float32),
        "b2": np.tile(biases[2][None, :], (P, 1)).astype(np.float32),
        "linw": np.tile(lin_w.reshape(1, DH), (P, 1)).astype(np.float32),
        "linb": np.tile(lin_b.reshape(1, 1), (P, 1)).astype(np.float32),
        "invcnt": invcnt,
    }
    for im in in_maps:
        im.update(params)

    cfg = dict(NL=NL, NT=NT, GP=GP, m_lo=tuple(m_lo), m_hi=tuple(m_hi),
               SM=SM, ncores=ncores)
    return cfg, in_maps


# ------------------------------------------------------------- program build

def build_program(cfg, use_f32r=False, reps=1, dup=None, no_cc=False):
    NL, NT, GP = cfg["NL"], cfg["NT"], cfg["GP"]
    SM, ncores = cfg["SM"], cfg["ncores"]
    m_lo, m_hi = cfg["m_lo"], cfg["m_hi"]
    NG = NL * ncores                     # padded-global node rows
    HALF = NG // 2
    MMDT = mybir.dt.float32r if use_f32r else F32

    nc = bacc.Bacc("TRN2", target_bir_lowering=False, debug=False,
                   num_devices=ncores, dynamic_dma_scratch_size=32768)
    # ---------------- I/O
    x_in = nc.dram_tensor("x_local", [NL, IN_CH], F32, kind="ExternalInput")
    idx16 = nc.dram_tensor("idx16", [P, 8 * SM], mybir.dt.int16,
                           kind="ExternalInput")
    dstloc = nc.dram_tensor("dstloc", [P * SM], F32, kind="ExternalInput")
    bshift = nc.dram_tensor("bshift", [NL], F32, kind="ExternalInput")
    gslot = nc.dram_tensor("gslot", [P, 1], I32, kind="ExternalInput")
    Waug = [nc.dram_tensor(f"W{l}aug", [IN_CH if l == 0 else DH, DW], F32,
                           kind="ExternalInput") for l in range(3)]
    bias = [nc.dram_tensor(f"b{l}", [P, DH], F32, kind="ExternalInput")
            for l in range(3)]
    linw = nc.dram_tensor("linw", [P, DH], F32, kind="ExternalInput")
    linb = nc.dram_tensor("linb", [P, 1], F32, kind="ExternalInput")
    invcnt = nc.dram_tensor("invcnt", [GP, 1], F32, kind="ExternalInput")
    y = nc.dram_tensor("y", [GP, 1], F32, kind="ExternalOutput")

    with tile.TileContext(nc) as tc:
        with tc.tile_pool(name="const", bufs=1) as cst, \
             tc.tile_pool(name="dram", bufs=1, space="DRAM") as dram, \
             tc.tile_pool(name="work", bufs=WORK_BUFS) as wk, \
             tc.tile_pool(name="gpool", bufs=GATHER_BUFS) as gp, \
             tc.tile_pool(name="psA", bufs=PSA_BUFS, space="PSUM") as psA, \
             tc.tile_pool(name="psB", bufs=PSB_BUFS, space="PSUM") as psB, \
             tc.tile_pool(name="psC", bufs=2, space="PSUM") as psC, \
             tc.tile_pool(name="psN", bufs=1, space="PSUM") as psN, \
             tc.tile_pool(name="psP", bufs=1, space="PSUM") as psP:

            # ---------------- DRAM intermediates
            hh_local = dram.tile([NL, DG], F32)
            ad_local = dram.tile([NL, HEADS], F32)
            hh_full = dram.tile([NG, DG], F32)
            helu = [dram.tile([NL, DH], F32, tag=f"helu{i}", name=f"helu{i}")
                    for i in range(2)]
            pool_loc = dram.tile([GP, DH], F32)
            pool_sum = dram.tile([GP, DH], F32)

            # ---------------- constants
            ident = cst.tile([P, P], MMDT)
            ident_f = cst.tile([P, P], F32)
            iota_i = cst.tile([P, P], I32)
            nc.gpsimd.iota(iota_i[:], pattern=[[1, P]], base=0,
                           channel_multiplier=0)
            iota_f = cst.tile([P, P], F32)
            nc.vector.tensor_copy(iota_f[:], iota_i[:])
            iota_ci = cst.tile([P, 1], I32)
            nc.gpsimd.iota(iota_ci[:], pattern=[[0, 1]], base=0,
                           channel_multiplier=1)
            iota_cf = cst.tile([P, 1], F32)
            nc.vector.tensor_copy(iota_cf[:], iota_ci[:])
            nc.vector.tensor_tensor(out=ident[:],
                                    in0=iota_cf[:].to_broadcast([P, P]),
                                    in1=iota_f[:], op=mybir.AluOpType.is_equal)
            nc.vector.tensor_tensor(out=ident_f[:],
                                    in0=iota_cf[:].to_broadcast([P, P]),
                                    in1=iota_f[:], op=mybir.AluOpType.is_equal)

            idx_all = cst.tile([P, 8 * SM], mybir.dt.int16)
            nc.sync.dma_start(idx_all[:], idx16[:, :])
            dst_all = cst.tile([P, SM], F32)
            nc.sync.dma_start(dst_all[:], dstloc[:].rearrange("(p j) -> p j", j=SM))

            W_t = []
            for l in range(3):
                cin = IN_CH if l == 0 else DH
                tiles = []
                for kk in range(cin // P):
                    t = cst.tile([P, DW], MMDT, tag=f"W{l}_{kk}")
                    nc.gpsimd.dma_start(t[:], Waug[l][kk * P:(kk + 1) * P, :])
                    tiles.append(t)
                W_t.append(tiles)
            bias_t = []
            for l in range(3):
                t = cst.tile([P, DH], F32, tag=f"bias{l}")
                nc.sync.dma_start(t[:], bias[l][:, :])
                bias_t.append(t)
            linw_t = cst.tile([P, DH], F32)
            nc.sync.dma_start(linw_t[:], linw[:, :])
            linb_t = cst.tile([P, 1], F32)
            nc.sync.dma_start(linb_t[:], linb[:, :])
            gslot_t = cst.tile([P, 1], I32)
            nc.sync.dma_start(gslot_t[:], gslot[:, :])
            # hh_local pad columns are never written by the node phase but are
            # AllGathered; zero them once so sim stays finite.
            zpad = cst.tile([P, DG - DA], F32)
            nc.gpsimd.memset(zpad[:], 0.0)
            for nt in range(NT):
                nc.sync.dma_start(hh_local[nt * P:(nt + 1) * P, DA:DG], zpad[:])

            # ---------------- phases
            def node_phase(l):
                """h_in (x or helu[l-1]) @ Waug_l -> hh_local, ad_local."""
                cin = IN_CH if l == 0 else DH
                src_d = x_in if l == 0 else helu[l - 1]
                for nt in range(NT):
                    in_t = wk.tile([P, cin], F32, tag="node_in")
                    nc.sync.dma_start(in_t[:], src_d[nt * P:(nt + 1) * P, :])
                    ps_o = psN.tile([P, DW], F32, space="PSUM", tag="node_mm")
                    for kk in range(cin // P):
                        trp = psC.tile([P, P], F32, space="PSUM", tag="trp")
                        nc.tensor.transpose(out=trp[:],
                                            in_=in_t[:, kk * P:(kk + 1) * P],
                                            identity=ident_f[:])
                        inT = wk.tile([P, P], MMDT, tag="node_inT")
                        nc.vector.tensor_copy(inT[:], trp[:])
                        nc.tensor.matmul(ps_o[:], lhsT=inT[:], rhs=W_t[l][kk][:],
                                         start=(kk == 0), stop=(kk == cin // P - 1))
                    hh_t = wk.tile([P, DW], F32, tag="node_hh")
                    nc.vector.tensor_copy(hh_t[:], ps_o[:])
                    nc.sync.dma_start(hh_local[nt * P:(nt + 1) * P, 0:DA],
                                      hh_t[:, 0:DA])
                    nc.sync.dma_start(ad_local[nt * P:(nt + 1) * P, :],
                                      hh_t[:, DA:DW])

            def all_gather_hh():
                if no_cc:
                    nc.sync.dma_start(hh_full[0:NL, :], hh_local[:, :])
                    return
                nc.gpsimd.collective_compute(
                    "AllGather", mybir.AluOpType.bypass,
                    ins=[hh_local[:, :].opt()], outs=[hh_full[:, :].opt()],
                    replica_groups=[list(range(ncores))])

            def edge_phase(l):
                last = (l == 2)
                if last:
                    pool_ps = psP.tile([P, DH], F32, space="PSUM", tag="pool")
                off = 0
                for t in range(NT):
                    mtot = m_lo[t] + m_hi[t]
                    ad_t = wk.tile([P, HEADS], MMDT, tag="ad")
                    nc.gpsimd.dma_start(ad_t[:], ad_local[t * P:(t + 1) * P, :])
                    acc = psA.tile([P, DA], F32, space="PSUM", tag="acc")
                    jglob = 0
                    for half, mh in ((0, m_lo[t]), (1, m_hi[t])):
                      base = HALF * half
                      j = 0
                      while j < mh:
                        kb = min(KB, mh - j)
                        co = off + j
                        g4 = gp.tile([P, KB * DG], F32, tag="hhg")
                        ohs = []
                        adg4 = psB.tile([P, KB * HEADS], F32, space="PSUM",
                                        tag="adg")
                        nc.gpsimd.dma_gather(
                            out_ap=g4[:, 0:kb * DG].rearrange(
                                "p (q d) -> p q d", q=kb),
                            in_ap=hh_full[base:base + HALF, :],
                            idxs_ap=idx_all[:, co * 8:(co + kb) * 8],
                            num_idxs=kb * P, num_idxs_reg=kb * P,
                            elem_size=DG)
                        for q in range(kb):
                            c = co + q
                            oh = wk.tile([P, P], MMDT, tag=f"oh{q}")
                            nc.vector.tensor_tensor(
                                out=oh[:],
                                in0=dst_all[:, c:c + 1].to_broadcast([P, P]),
                                in1=iota_f[:], op=mybir.AluOpType.is_equal)
                            ohs.append(oh)
                            if E_ATT:
                                trp = psC.tile([P, P], MMDT, space="PSUM", tag="trp")
                                nc.tensor.transpose(out=trp[:], in_=oh[:],
                                                    identity=ident[:])
                                ohT = wk.tile([P, P], MMDT, tag="ohT")
                                nc.vector.tensor_copy(ohT[:], trp[:])
                                nc.tensor.matmul(
                                    adg4[:, q * HEADS:(q + 1) * HEADS], lhsT=ohT[:],
                                    rhs=ad_t[:], start=True, stop=True)
                            elif q == 0:
                                nc.vector.tensor_copy(adg4[:, 0:KB * HEADS],
                                                      g4[:, 0:KB * HEADS])
                        rhs4 = gp.tile([P, KB * DA], MMDT, tag="rhs")
                        e4 = wk.tile([P, KB * HEADS], F32, tag="e")
                        # e = as_g + ad_g (batched over the kb gathers)
                        nc.vector.tensor_add(
                            e4[:, 0:kb * HEADS].rearrange(
                                "p (q h) -> p q h", q=kb),
                            g4[:, 0:kb * DG].rearrange(
                                "p (q d) -> p q d", q=kb)[:, :, DH:DA],
                            adg4[:, 0:kb * HEADS].rearrange(
                                "p (q h) -> p q h", q=kb))
                        nc.vector.scalar_tensor_tensor(
                            out=e4[:, 0:kb * HEADS], in0=e4[:, 0:kb * HEADS],
                            scalar=NEG, in1=e4[:, 0:kb * HEADS],
                            op0=mybir.AluOpType.mult, op1=mybir.AluOpType.max)
                        nc.scalar.activation(
                            rhs4[:, 0:kb * DA].rearrange(
                                "p (q d) -> p q d", q=kb)[:, :, DH:DA],
                            e4[:, 0:kb * HEADS].rearrange(
                                "p (q h) -> p q h", q=kb),
                            mybir.ActivationFunctionType.Exp)
                        if E_MUL:
                            nc.vector.tensor_mul(
                            rhs4[:, 0:kb * DA].rearrange(
                                "p (q d) -> p q d", q=kb)[:, :, 0:DH].rearrange(
                                "p q (h c) -> p q h c", h=HEADS),
                            g4[:, 0:kb * DG].rearrange(
                                "p (q d) -> p q d", q=kb)[:, :, 0:DH].rearrange(
                                "p q (h c) -> p q h c", h=HEADS),
                            rhs4[:, 0:kb * DA].rearrange(
                                "p (q d) -> p q d", q=kb)[:, :, DH:DA][
                                :, :, :, None].to_broadcast(
                                [P, kb, HEADS, HID]))
                        if E_SMM:
                            for q in range(kb):
                                nc.tensor.matmul(
                                    acc[:], lhsT=ohs[q][:],
                                    rhs=rhs4[:, q * DA:(q + 1) * DA],
                                    start=(jglob + q == 0),
                                    stop=(jglob + q == mtot - 1))
                        elif jglob == 0:
                            nc.tensor.matmul(acc[:], lhsT=ohs[0][:],
                                             rhs=rhs4[:, 0:DA],
                                             start=True, stop=True)
                        j += kb
                        jglob += kb
                      off += mh
                    # epilogue: out = elu(num/den + bias)
                    inv_t = wk.tile([P, HEADS], F32, tag="inv")
                    nc.vector.reciprocal(inv_t[:], acc[:, DH:DA])
                    h0 = wk.tile([P, DH], F32, tag="h0")
                    nc.vector.tensor_mul(
                        h0[:].rearrange("p (h c) -> p h c", h=HEADS),
                        acc[:, 0:DH].rearrange("p (h c) -> p h c", h=HEADS),
                        inv_t[:, :, None].to_broadcast([P, HEADS, HID]))
                    nc.vector.tensor_add(h0[:], h0[:], bias_t[l][:])
                    tm = wk.tile([P, DH], F32, tag="tm")
                    nc.vector.tensor_scalar_min(tm[:], h0[:], 0.0)
                    nc.scalar.activation(tm[:], tm[:],
                                         mybir.ActivationFunctionType.Exp)
                    out_t = wk.tile([P, DH], F32, tag="hout")
                    nc.vector.scalar_tensor_tensor(
                        out=out_t[:], in0=h0[:], scalar=0.0, in1=tm[:],
                        op0=mybir.AluOpType.max, op1=mybir.AluOpType.add)
                    nc.vector.tensor_scalar_add(out_t[:], out_t[:], -1.0)
                    if not last:
                        nc.sync.dma_start(helu[l][t * P:(t + 1) * P, :], out_t[:])
                    else:
                        gcol = wk.tile([P, 1], F32, tag="gcol")
                        nc.sync.dma_start(gcol[:], bshift[t * P:(t + 1) * P, None])
                        ohp = wk.tile([P, P], F32, tag="ohp")
                        nc.vector.tensor_tensor(
                            out=ohp[:], in0=gcol[:, 0:1].to_broadcast([P, P]),
                            in1=iota_f[:], op=mybir.AluOpType.is_equal)
                        nc.tensor.matmul(pool_ps[:], lhsT=ohp[:], rhs=out_t[:],
                                         start=(t == 0), stop=(t == NT - 1))

                if last:
                    # zero pool_loc then scatter local slots
                    zt = wk.tile([P, DH], F32, tag="zero")
                    nc.gpsimd.memset(zt[:], 0.0)
                    for b in range(GP // P):
                        nc.sync.dma_start(pool_loc[b * P:(b + 1) * P, :], zt[:])
                    pl = wk.tile([P, DH], F32, tag="plocal")
                    nc.vector.tensor_copy(pl[:], pool_ps[:])
                    nc.gpsimd.indirect_dma_start(
                        out=pool_loc[:, :],
                        out_offset=bass.IndirectOffsetOnAxis(
                            ap=gslot_t[:, 0:1], axis=0),
                        in_=pl[:, :], in_offset=None,
                        bounds_check=GP - 1, oob_is_err=False)

            # ---------------- run the layers
            for _rep in range(reps):
                for l in range(3):
                    if not SKIP_NODE:
                        node_phase(l)
                        if dup == "node":
                            node_phase(l)
                    if not SKIP_AG:
                        all_gather_hh()
                        if dup == "ag":
                            all_gather_hh()
                    if not SKIP_EDGE:
                        edge_phase(l)
                        if dup == "edge":
                            edge_phase(l)

                if no_cc:
                    nc.sync.dma_start(pool_sum[:, :], pool_loc[:, :])
                else:
                    nc.gpsimd.collective_compute(
                        "AllReduce", mybir.AluOpType.add,
                        ins=[pool_loc[:, :].opt()], outs=[pool_sum[:, :].opt()],
                        replica_groups=[list(range(ncores))])

            # final linear: y = (pool_sum * invcnt) @ lin_w + lin_b
            for b in range(GP // P):
                pt = wk.tile([P, DH], F32, tag="psum_t")
                nc.sync.dma_start(pt[:], pool_sum[b * P:(b + 1) * P, :])
                ic = wk.tile([P, 1], F32, tag="ic")
                nc.sync.dma_start(ic[:], invcnt[b * P:(b + 1) * P, :])
                mulw = wk.tile([P, DH], F32, tag="mulw")
                nc.vector.tensor_mul(mulw[:], pt[:], linw_t[:])
                rs = wk.tile([P, 1], F32, tag="rs")
                nc.vector.reduce_sum(rs[:], mulw[:], axis=mybir.AxisListType.X)
                nc.vector.tensor_mul(rs[:], rs[:], ic[:])
                nc.vector.tensor_add(rs[:], rs[:], linb_t[:])
                nc.sync.dma_start(y[b * P:(b + 1) * P, :], rs[:])

    nc.compile()
    return nc


# ------------------------------------------------------------------- runner

class SpmdRunner:
    def __init__(self, nc, n_cores):
        import jax
        from jax.sharding import Mesh, PartitionSpec
        from jax.experimental.shard_map import shard_map
        from concourse.bass2jax import (
            _bass_exec_p, install_neuronx_cc_hook, partition_id_tensor)
        self.jax = jax
        install_neuronx_cc_hook()
        self.nc = nc
        self.n_cores = n_cores
        partition_name = (nc.partition_id_tensor.name
                          if nc.partition_id_tensor else None)
        in_names, out_names, out_avals, zero_outs = [], [], [], []
        for alloc in nc.m.functions[0].allocations:
            if not isinstance(alloc, mybir.MemoryLocationSet):
                continue
            name = alloc.memorylocations[0].name
            if alloc.kind == "ExternalInput":
                if name != partition_name and name != (
                        nc.dbg_addr.name if nc.dbg_addr else None):
                    in_names.append(name)
            elif alloc.kind == "ExternalOutput":
                out_names.append(name)
                shape = tuple(alloc.tensor_shape)
                dtype = mybir.dt.np(alloc.dtype)
                out_avals.append(jax.core.ShapedArray(shape, dtype))
                zero_outs.append(np.zeros(shape, dtype))
        self.in_names, self.out_names = in_names, out_names
        self.out_avals, self.zero_outs = out_avals, zero_outs
        n_params = len(in_names)
        all_in_names = list(in_names) + list(out_names)
        has_dbg = nc.dbg_addr is not None
        if has_dbg:
            all_in_names.append(nc.dbg_addr.name)
        if partition_name is not None:
            all_in_names.append(partition_name)

        def _body(*args):
            operands = list(args)
            if has_dbg:
                operands.append(jax.numpy.zeros((1, 2), jax.numpy.uint32))
            if partition_name is not None:
                operands.append(partition_id_tensor())
            outs = _bass_exec_p.bind(
                *operands, out_avals=tuple(out_avals),
                in_names=tuple(all_in_names), out_names=tuple(out_names),
                lowering_input_output_aliases=(),
                sim_require_finite=False, sim_require_nnan=False, nc=nc)
            return tuple(outs)

        devices = jax.devices()[:n_cores]
        assert len(devices) == n_cores
        mesh = Mesh(np.asarray(devices), ("core",))
        in_specs = (PartitionSpec("core"),) * (n_params + len(out_names))
        out_specs = (PartitionSpec("core"),) * len(out_names)
        self.fn = jax.jit(
            shard_map(_body, mesh=mesh, in_specs=in_specs,
                      out_specs=out_specs, check_rep=False),
            keep_unused=True)

    def prepare(self, in_maps):
        per_core = [[np.ascontiguousarray(m[nm]) for nm in self.in_names]
                    for m in in_maps]
        concat_in = [
            np.concatenate([per_core[c][i] for c in range(self.n_cores)], axis=0)
            for i in range(len(self.in_names))]
        concat_zero = [
            np.zeros((self.n_cores * z.shape[0], *z.shape[1:]), z.dtype)
            for z in self.zero_outs]
        args = [self.jax.device_put(a) for a in concat_in + concat_zero]
        for a in args:
            a.block_until_ready()
        return args

    def run(self, args):
        outs = self.fn(*args)
        self.jax.block_until_ready(outs)
        return outs

    def results(self, outs):
        res = []
        for c in range(self.n_cores):
            m = {}
            for i, nm in enumerate(self.out_names):
                m[nm] = np.asarray(outs[i]).reshape(
                    self.n_cores, *self.out_avals[i].shape)[c]
            res.append(m)
        return res


# -------------------------------------------------------------------- kernel

_CACHE = {}

N_FULL, E_FULL, G_FULL, NCORES = 50000, 800000, 512, 8
USE_F32R = True


def kernel(x, edge_index, batch,
           W0, a_src0, a_dst0, bias0,
           W1, a_src1, a_dst1, bias1,
           W2, a_src2, a_dst2, bias2,
           lin_w, lin_b):
    x = np.asarray(x, np.float32)
    edge_index = np.asarray(edge_index, np.int64)
    batch = np.asarray(batch, np.int64)
    N, E, G = x.shape[0], edge_index.shape[1], G_FULL

    cfg, in_maps = host_prep(
        x, edge_index, batch,
        [np.asarray(W0, np.float32), np.asarray(W1, np.float32),
         np.asarray(W2, np.float32)],
        [np.asarray(a_src0, np.float32), np.asarray(a_src1, np.float32),
         np.asarray(a_src2, np.float32)],
        [np.asarray(a_dst0, np.float32), np.asarray(a_dst1, np.float32),
         np.asarray(a_dst2, np.float32)],
        [np.asarray(bias0, np.float32), np.asarray(bias1, np.float32),
         np.asarray(bias2, np.float32)],
        np.asarray(lin_w, np.float32), np.asarray(lin_b, np.float32),
        N, E, G, NCORES)

    key = (cfg["NL"], cfg["NT"], cfg["GP"], cfg["m_lo"], cfg["m_hi"],
           cfg["SM"], cfg["ncores"], USE_F32R)
    if key not in _CACHE:
        nc = build_program(cfg, use_f32r=USE_F32R)
        _CACHE[key] = (nc, SpmdRunner(nc, NCORES))
    nc, runner = _CACHE[key]

    args = runner.prepare(in_maps)
    outs = runner.run(args)
    res = runner.results(outs)
    return res[0]["y"][:G].astype(np.float32)



# revision 4
# speedup vs baseline: 2.3617x; 2.3617x over previous
"""3-layer GAT (8 heads x 32 hid, PyG GATConv semantics w/ self-loops) +
global mean pool + linear, distributed over 8 Trainium2 NeuronCores.

v2 strategy: nodes partitioned into 8 contiguous ranges (dst-owner);
non-self-loop edges live with their dst core, bucketed per (dst tile,
src chunk), sorted by src. All node features flow through a bf16 table
hh = [h | alpha_src] (DG=384-wide rows, 768B — dma_gather elem granularity).

Per layer:
  - node transform is FUSED into the previous layer's edge epilogue
    (layer 0 has a standalone node sweep over x).
  - hh is AllGathered in TWO chunks (tiles 0-24 -> hh_A, tiles 25-48 ->
    hh_B) fired as soon as the respective node tiles finish, so the
    collectives overlap edge-phase compute.
  - edge phase runs two passes: pass-lo processes edges whose src is in
    chunk A (overlapping the in-flight AG of chunk B), drains partial
    [num|den] per dst tile to SBUF; pass-hi adds chunk-B edges, then the
    epilogue computes out = elu(num/den + bias) and immediately runs the
    next layer's node matmul for that tile.
  - self-loop contributions are computed locally (no gather) and seed the
    PSUM accumulator, which also guarantees den>0 for padded rows.
  - segment softmax is algebraically folded: out = sum(ex*h)/sum(ex),
    no max subtraction (attention logits are O(1)-bounded).
  - gathers use dma_gather with 512 rows/instr, alternating SWDGE queues
    (descriptor generation parallelizes across queues).
Per-graph mean-pool partials are scattered into a 512-row buffer and
AllReduced; the final linear layer is tiny.

Self-contained: hardcodes the problem shapes; host-side preprocessing uses
only graph structure (edge_index, batch) and parameter repacking.
"""
import numpy as np
import ml_dtypes

import concourse.bass as bass
import concourse.bacc as bacc
import concourse.mybir as mybir
import concourse.tile as tile

P = 128
KB = 4                    # 128-edge groups per dma_gather
NQ = 4                    # SWDGE queues for gather descriptor generation
WORK_BUFS = 8
GATHER_BUFS = 4
HEADS, HID = 8, 32
DH = HEADS * HID          # 256
DA = DH + HEADS           # 264 = h | alpha_src
DW = DH + 2 * HEADS       # 272 = W | W@Asrc | W@Adst
DG = 384                  # bf16 row padded so DG*2B % 256B == 0
IN_CH = 128
NEG = 0.2
F32 = mybir.dt.float32
I32 = mybir.dt.int32
BF16 = mybir.dt.bfloat16
EXP = mybir.ActivationFunctionType.Exp


# ----------------------------------------------------------------- host prep

def host_prep(x, edge_index, batch, Ws, a_srcs, a_dsts, biases, lin_w, lin_b,
              N, E, G, ncores):
    """Build per-core input maps + the (core-uniform) program config."""
    nl = N // ncores
    assert nl * ncores == N
    NL = ((nl + P - 1) // P) * P               # padded local nodes (6272)
    NT = NL // P                               # dst tiles per core (49)
    TA = (NT + 1) // 2                         # chunk-A tiles (25)
    CHA, CHB = TA * P, (NT - TA) * P           # 3200 / 3072 local rows
    GP = ((G + P - 1) // P) * P
    assert 8 * CHA <= 32768 and 8 * CHB <= 32768

    src = np.asarray(edge_index[0])
    dst = np.asarray(edge_index[1])
    core_of = dst // nl
    dloc_all = dst - core_of * nl

    s_core = src // nl
    s_loc = src - s_core * nl
    s_half = (s_loc >= CHA).astype(np.int64)
    gsrc = np.where(s_half == 0, s_core * CHA + s_loc,
                    s_core * CHB + (s_loc - CHA))

    # per-core, per-tile, per-half edge lists sorted by gsrc
    per_core = []
    cnt = np.zeros((ncores, NT, 2), np.int64)
    for k in range(ncores):
        m = core_of == k
        g_k, d_k, h_k = gsrc[m], dloc_all[m], s_half[m]
        order = np.lexsort((g_k, h_k, d_k // P))
        g_k, d_k, h_k = g_k[order], d_k[order], h_k[order]
        t_k = d_k // P
        tiles = []
        for t in range(NT):
            mt = t_k == t
            g_t, d_t, h_t = g_k[mt], d_k[mt] - t * P, h_k[mt]
            lo = (g_t[h_t == 0], d_t[h_t == 0])
            hi = (g_t[h_t == 1], d_t[h_t == 1])
            tiles.append((lo, hi))
            cnt[k, t, 0] = len(lo[0])
            cnt[k, t, 1] = len(hi[0])
        per_core.append(tiles)
    cmax = cnt.max(axis=0)                                  # [NT, 2]
    m_lo = [int(v) for v in np.ceil(cmax[:, 0] / P).astype(np.int64)]
    m_hi = [int(v) for v in np.ceil(cmax[:, 1] / P).astype(np.int64)]
    SM = int(sum(m_lo) + sum(m_hi))
    # column offset of each tile's run (all-lo first, then all-hi)
    col_lo, col_hi = [], []
    c = 0
    for t in range(NT):
        col_lo.append(c)
        c += m_lo[t]
    for t in range(NT):
        col_hi.append(c)
        c += m_hi[t]

    in_maps = []
    batch = np.asarray(batch, np.int64)
    for k in range(ncores):
        dstflat = np.full((P, SM), 999.0, np.float32)
        idx16 = np.zeros((16, 8 * SM), np.int16)
        for t in range(NT):
            for half, mh, c0 in ((0, m_lo[t], col_lo[t]),
                                 (1, m_hi[t], col_hi[t])):
                if mh == 0:
                    continue
                g_t, d_t = per_core[k][t][half]
                L = mh * P
                gi = np.zeros(L, np.int64)
                gi[:len(g_t)] = g_t
                dd = np.full(L, 999.0, np.float32)
                dd[:len(d_t)] = d_t
                ii = np.arange(L)
                dstflat[ii % P, c0 + ii // P] = dd
                j = 0
                while j < mh:
                    kb = min(KB, mh - j)
                    arr = gi[j * P:(j + kb) * P]
                    idx16[:, (c0 + j) * 8:(c0 + j + kb) * 8] = \
                        arr.reshape(-1, 16).T.astype(np.int16)
                    j += kb

        bl = batch[k * nl:(k + 1) * nl]
        g_base = int(bl[0])
        bshift = np.full((NL,), 999.0, np.float32)
        bshift[:nl] = (bl - g_base).astype(np.float32)
        gslot = np.arange(g_base, g_base + P, dtype=np.int64)
        gslot = np.where(gslot < G, gslot, 99999).astype(np.int32)[:, None]

        xk = np.zeros((NL, IN_CH), np.float32)
        xk[:nl] = x[k * nl:(k + 1) * nl]

        im = {
            "x_local": xk.astype(ml_dtypes.bfloat16),
            "idx16": np.tile(idx16, (8, 1)),
            "dstloc": dstflat.astype(ml_dtypes.bfloat16),
            "bshift": bshift.reshape(NT, P).T.copy(),     # [P, NT]
            "gslot": gslot,
        }
        in_maps.append(im)

    # replicated parameters
    def aug(W, a_s, a_d):
        As = np.zeros((DH, HEADS), np.float64)
        Ad = np.zeros((DH, HEADS), np.float64)
        for h in range(HEADS):
            As[h * HID:(h + 1) * HID, h] = a_s[h]
            Ad[h * HID:(h + 1) * HID, h] = a_d[h]
        W = W.astype(np.float64)
        out = np.concatenate([W, W @ As, W @ Ad], axis=1)
        return out.astype(ml_dtypes.bfloat16)

    cnts = np.bincount(batch, minlength=G).astype(np.float32)
    invcnt = np.zeros((GP, 1), np.float32)
    invcnt[:G, 0] = 1.0 / np.maximum(cnts, 1.0)
    params = {
        "W0aug": aug(Ws[0], a_srcs[0], a_dsts[0]),
        "W1aug": aug(Ws[1], a_srcs[1], a_dsts[1]),
        "W2aug": aug(Ws[2], a_srcs[2], a_dsts[2]),
        "b0": np.tile(biases[0][None, :], (P, 1)).astype(np.float32),
        "b1": np.tile(biases[1][None, :], (P, 1)).astype(np.float32),
        "b2": np.tile(biases[2][None, :], (P, 1)).astype(np.float32),
        "linw": np.tile(lin_w.reshape(1, DH), (P, 1)).astype(np.float32),
        "linb": np.tile(lin_b.reshape(1, 1), (P, 1)).astype(np.float32),
        "invcnt": invcnt,
    }
    for im in in_maps:
        im.update(params)

    cfg = dict(NL=NL, NT=NT, TA=TA, GP=GP, m_lo=tuple(m_lo), m_hi=tuple(m_hi),
               col_lo=tuple(col_lo), col_hi=tuple(col_hi), SM=SM,
               ncores=ncores)
    return cfg, in_maps


# ------------------------------------------------------------- program build

def build_program(cfg, reps=1, dup=None, no_cc=False):
    NL, NT, TA, GP = cfg["NL"], cfg["NT"], cfg["TA"], cfg["GP"]
    SM, ncores = cfg["SM"], cfg["ncores"]
    m_lo, m_hi = cfg["m_lo"], cfg["m_hi"]
    col_lo, col_hi = cfg["col_lo"], cfg["col_hi"]
    CHA, CHB = TA * P, (NT - TA) * P

    nc = bacc.Bacc("TRN2", target_bir_lowering=False, debug=False,
                   num_devices=ncores, dynamic_dma_scratch_size=32768,
                   num_swdge_queues=NQ)
    # ---------------- I/O
    x_in = nc.dram_tensor("x_local", [NL, IN_CH], BF16, kind="ExternalInput")
    idx16 = nc.dram_tensor("idx16", [P, 8 * SM], mybir.dt.int16,
                           kind="ExternalInput")
    dstloc = nc.dram_tensor("dstloc", [P, SM], BF16, kind="ExternalInput")
    bshift = nc.dram_tensor("bshift", [P, NT], F32, kind="ExternalInput")
    gslot = nc.dram_tensor("gslot", [P, 1], I32, kind="ExternalInput")
    Waug = [nc.dram_tensor(f"W{l}aug", [IN_CH if l == 0 else DH, DW], BF16,
                           kind="ExternalInput") for l in range(3)]
    bias = [nc.dram_tensor(f"b{l}", [P, DH], F32, kind="ExternalInput")
            for l in range(3)]
    linw = nc.dram_tensor("linw", [P, DH], F32, kind="ExternalInput")
    linb = nc.dram_tensor("linb", [P, 1], F32, kind="ExternalInput")
    invcnt = nc.dram_tensor("invcnt", [GP, 1], F32, kind="ExternalInput")
    y = nc.dram_tensor("y", [GP, 1], F32, kind="ExternalOutput")

    with tile.TileContext(nc) as tc:
        with tc.tile_pool(name="const", bufs=1) as cst, \
             tc.tile_pool(name="dram", bufs=1, space="DRAM") as dram, \
             tc.tile_pool(name="work", bufs=WORK_BUFS) as wk, \
             tc.tile_pool(name="gpool", bufs=GATHER_BUFS) as gp, \
             tc.tile_pool(name="psA", bufs=2, space="PSUM") as psA, \
             tc.tile_pool(name="psB", bufs=2, space="PSUM") as psB, \
             tc.tile_pool(name="psC", bufs=1, space="PSUM") as psC, \
             tc.tile_pool(name="psN", bufs=1, space="PSUM") as psN, \
             tc.tile_pool(name="psP", bufs=1, space="PSUM") as psP:

            # ---------------- DRAM intermediates
            hh_local = dram.tile([NL, DG], BF16)
            hh_A = dram.tile([ncores * CHA, DG], BF16)
            hh_B = dram.tile([ncores * CHB, DG], BF16)
            pool_loc = dram.tile([GP, DH], F32)
            pool_sum = dram.tile([GP, DH], F32)

            # ---------------- constants
            iota_i = cst.tile([P, P], I32)
            nc.gpsimd.iota(iota_i[:], pattern=[[1, P]], base=0,
                           channel_multiplier=0)
            iota_b = cst.tile([P, P], BF16)
            nc.vector.tensor_copy(iota_b[:], iota_i[:])
            iota_f = cst.tile([P, P], F32)
            nc.vector.tensor_copy(iota_f[:], iota_i[:])
            iota_ci = cst.tile([P, 1], I32)
            nc.gpsimd.iota(iota_ci[:], pattern=[[0, 1]], base=0,
                           channel_multiplier=1)
            iota_cf = cst.tile([P, 1], F32)
            nc.vector.tensor_copy(iota_cf[:], iota_ci[:])
            ident_b = cst.tile([P, P], BF16)
            nc.vector.tensor_tensor(out=ident_b[:],
                                    in0=iota_cf[:].to_broadcast([P, P]),
                                    in1=iota_f[:], op=mybir.AluOpType.is_equal)

            idx_all = cst.tile([P, 8 * SM], mybir.dt.int16)
            nc.sync.dma_start(idx_all[:], idx16[:, :])
            dst_all = cst.tile([P, SM], BF16)
            nc.sync.dma_start(dst_all[:], dstloc[:, :])
            bsh_t = cst.tile([P, NT], F32)
            nc.sync.dma_start(bsh_t[:], bshift[:, :])

            W_t = []
            for l in range(3):
                cin = IN_CH if l == 0 else DH
                tiles = []
                for kk in range(cin // P):
                    t = cst.tile([P, DW], BF16, tag=f"W{l}_{kk}")
                    nc.sync.dma_start(t[:], Waug[l][kk * P:(kk + 1) * P, :])
                    tiles.append(t)
                W_t.append(tiles)
            bias_t = []
            for l in range(3):
                t = cst.tile([P, DH], F32, tag=f"bias{l}")
                nc.sync.dma_start(t[:], bias[l][:, :])
                bias_t.append(t)
            linw_t = cst.tile([P, DH], F32)
            nc.sync.dma_start(linw_t[:], linw[:, :])
            linb_t = cst.tile([P, 1], F32)
            nc.sync.dma_start(linb_t[:], linb[:, :])
            gslot_t = cst.tile([P, 1], I32)
            nc.sync.dma_start(gslot_t[:], gslot[:, :])

            # zero hh_local pad columns once (they ride along in the AG)
            zpad = cst.tile([P, DG - DA], BF16)
            nc.gpsimd.memset(zpad[:], 0.0)
            for nt in range(NT):
                nc.sync.dma_start(hh_local[nt * P:(nt + 1) * P, DA:DG],
                                  zpad[:])

            # per-layer alpha_src/alpha_dst for local nodes, kept in SBUF
            as_sb = cst.tile([P, NT * HEADS], F32)
            ad_sb = cst.tile([P, NT * HEADS], F32)
            # pass-lo partial [num|den] per dst tile
            part_sb = cst.tile([P, NT * DA], F32)

            gq = [0]  # gather queue round-robin counter

            # ---------------- helpers
            def node_tile(l, t, src_bf):
                """src_bf [P, cin] bf16 -> hh_local[t], as_sb/ad_sb col t."""
                cin = IN_CH if l == 0 else DH
                ps_o = psN.tile([P, DW], F32, space="PSUM", tag="node_mm")
                for kk in range(cin // P):
                    trp = psC.tile([P, P], BF16, space="PSUM", tag="trp")
                    nc.tensor.transpose(out=trp[:],
                                        in_=src_bf[:, kk * P:(kk + 1) * P],
                                        identity=ident_b[:])
                    inT = wk.tile([P, P], BF16, tag="node_inT")
                    nc.vector.tensor_copy(inT[:], trp[:])
                    nc.tensor.matmul(ps_o[:], lhsT=inT[:], rhs=W_t[l][kk][:],
                                     start=(kk == 0), stop=(kk == cin // P - 1))
                hh_t = wk.tile([P, DA], BF16, tag="node_hh")
                nc.vector.tensor_copy(hh_t[:], ps_o[:, 0:DA])
                nc.sync.dma_start(hh_local[t * P:(t + 1) * P, 0:DA], hh_t[:])
                nc.vector.tensor_copy(as_sb[:, t * HEADS:(t + 1) * HEADS],
                                      ps_o[:, DH:DH + HEADS])
                nc.vector.tensor_copy(ad_sb[:, t * HEADS:(t + 1) * HEADS],
                                      ps_o[:, DH + HEADS:DW])

            def all_gather(chunk):
                if no_cc:
                    if chunk == 0:
                        nc.sync.dma_start(hh_A[0:CHA, :], hh_local[0:CHA, :])
                    else:
                        nc.sync.dma_start(hh_B[0:CHB, :],
                                          hh_local[CHA:NL, :])
                    return
                if chunk == 0:
                    nc.gpsimd.collective_compute(
                        "AllGather", mybir.AluOpType.bypass,
                        ins=[hh_local[0:CHA, :].opt()],
                        outs=[hh_A[:, :].opt()],
                        replica_groups=[list(range(ncores))])
                else:
                    nc.gpsimd.collective_compute(
                        "AllGather", mybir.AluOpType.bypass,
                        ins=[hh_local[CHA:NL, :].opt()],
                        outs=[hh_B[:, :].opt()],
                        replica_groups=[list(range(ncores))])

            def edge_groups(t, half, acc, start):
                """Process the gather groups of (tile t, half). Returns True
                if any matmul was issued (acc live)."""
                mh = (m_lo if half == 0 else m_hi)[t]
                c0 = (col_lo if half == 0 else col_hi)[t]
                src_d = hh_A if half == 0 else hh_B
                ad_t = wk.tile([P, HEADS], BF16, tag="ad")
                nc.vector.tensor_copy(ad_t[:],
                                      ad_sb[:, t * HEADS:(t + 1) * HEADS])
                j = 0
                while j < mh:
                    kb = min(KB, mh - j)
                    co = c0 + j
                    g4 = gp.tile([P, KB * DG], BF16, tag="hhg")
                    adg4 = psB.tile([P, KB * HEADS], F32, space="PSUM",
                                    tag="adg")
                    nc.gpsimd.dma_gather(
                        out_ap=g4[:, 0:kb * DG].rearrange(
                            "p (q d) -> p q d", q=kb),
                        in_ap=src_d[:, :],
                        idxs_ap=idx_all[:, co * 8:(co + kb) * 8],
                        num_idxs=kb * P, num_idxs_reg=kb * P,
                        elem_size=DG, queue_num=gq[0] % NQ)
                    gq[0] += 1
                    ohs = []
                    for q in range(kb):
                        c = co + q
                        oh = wk.tile([P, P], BF16, tag=f"oh{q}")
                        nc.vector.tensor_tensor(
                            out=oh[:],
                            in0=dst_all[:, c:c + 1].to_broadcast([P, P]),
                            in1=iota_b[:], op=mybir.AluOpType.is_equal)
                        ohs.append(oh)
                        trp = psC.tile([P, P], BF16, space="PSUM", tag="trp")
                        nc.tensor.transpose(out=trp[:], in_=oh[:],
                                            identity=ident_b[:])
                        ohT = wk.tile([P, P], BF16, tag="ohT")
                        nc.vector.tensor_copy(ohT[:], trp[:])
                        nc.tensor.matmul(
                            adg4[:, q * HEADS:(q + 1) * HEADS], lhsT=ohT[:],
                            rhs=ad_t[:], start=True, stop=True)
                    rhs4 = gp.tile([P, KB * DA], BF16, tag="rhs")
                    e4 = wk.tile([P, KB * HEADS], F32, tag="e")
                    nc.vector.tensor_add(
                        e4[:, 0:kb * HEADS].rearrange(
                            "p (q h) -> p q h", q=kb),
                        g4[:, 0:kb * DG].rearrange(
                            "p (q d) -> p q d", q=kb)[:, :, DH:DA],
                        adg4[:, 0:kb * HEADS].rearrange(
                            "p (q h) -> p q h", q=kb))
                    nc.vector.scalar_tensor_tensor(
                        out=e4[:, 0:kb * HEADS], in0=e4[:, 0:kb * HEADS],
                        scalar=NEG, in1=e4[:, 0:kb * HEADS],
                        op0=mybir.AluOpType.mult, op1=mybir.AluOpType.max)
                    nc.scalar.activation(
                        rhs4[:, 0:kb * DA].rearrange(
                            "p (q d) -> p q d", q=kb)[:, :, DH:DA],
                        e4[:, 0:kb * HEADS].rearrange(
                            "p (q h) -> p q h", q=kb), EXP)
                    nc.vector.tensor_mul(
                        rhs4[:, 0:kb * DA].rearrange(
                            "p (q d) -> p q d", q=kb)[:, :, 0:DH].rearrange(
                            "p q (h c) -> p q h c", h=HEADS),
                        g4[:, 0:kb * DG].rearrange(
                            "p (q d) -> p q d", q=kb)[:, :, 0:DH].rearrange(
                            "p q (h c) -> p q h c", h=HEADS),
                        rhs4[:, 0:kb * DA].rearrange(
                            "p (q d) -> p q d", q=kb)[:, :, DH:DA][
                            :, :, :, None].to_broadcast(
                            [P, kb, HEADS, HID]))
                    for q in range(kb):
                        nc.tensor.matmul(
                            acc[:], lhsT=ohs[q][:],
                            rhs=rhs4[:, q * DA:(q + 1) * DA],
                            start=(start and j == 0 and q == 0),
                            stop=(j + kb >= mh and q == kb - 1))
                    j += kb
                return mh > 0

            def self_loop(t, acc):
                """Seed acc with the self-loop term (start=True matmul)."""
                hsrc = wk.tile([P, DH], BF16, tag="hself")
                nc.sync.dma_start(hsrc[:], hh_local[t * P:(t + 1) * P, 0:DH])
                es = wk.tile([P, HEADS], F32, tag="eself")
                nc.vector.tensor_add(es[:],
                                     as_sb[:, t * HEADS:(t + 1) * HEADS],
                                     ad_sb[:, t * HEADS:(t + 1) * HEADS])
                nc.vector.scalar_tensor_tensor(
                    out=es[:], in0=es[:], scalar=NEG, in1=es[:],
                    op0=mybir.AluOpType.mult, op1=mybir.AluOpType.max)
                rhs_s = wk.tile([P, DA], BF16, tag="rhs_s")
                nc.scalar.activation(rhs_s[:, DH:DA], es[:], EXP)
                nc.vector.tensor_mul(
                    rhs_s[:, 0:DH].rearrange("p (h c) -> p h c", h=HEADS),
                    hsrc[:].rearrange("p (h c) -> p h c", h=HEADS),
                    rhs_s[:, DH:DA][:, :, None].to_broadcast([P, HEADS, HID]))
                nc.tensor.matmul(acc[:], lhsT=ident_b[:], rhs=rhs_s[:],
                                 start=True, stop=(m_lo[t] == 0))

            def epilogue(l, t, sum_t, pool_ps):
                """sum_t [P, DA] f32 -> out bf16; fused next-layer node mm."""
                inv_t = wk.tile([P, HEADS], F32, tag="inv")
                nc.vector.reciprocal(inv_t[:], sum_t[:, DH:DA])
                h0 = wk.tile([P, DH], F32, tag="h0")
                nc.vector.tensor_mul(
                    h0[:].rearrange("p (h c) -> p h c", h=HEADS),
                    sum_t[:, 0:DH].rearrange("p (h c) -> p h c", h=HEADS),
                    inv_t[:, :, None].to_broadcast([P, HEADS, HID]))
                nc.vector.tensor_add(h0[:], h0[:], bias_t[l][:])
                tm = wk.tile([P, DH], F32, tag="tm")
                nc.vector.tensor_scalar_min(tm[:], h0[:], 0.0)
                nc.scalar.activation(tm[:], tm[:], EXP)
                out_t = wk.tile([P, DH], BF16, tag="hout")
                nc.vector.scalar_tensor_tensor(
                    out=out_t[:], in0=h0[:], scalar=0.0, in1=tm[:],
                    op0=mybir.AluOpType.max, op1=mybir.AluOpType.add)
                nc.vector.tensor_scalar_add(out_t[:], out_t[:], -1.0)
                if l < 2:
                    node_tile(l + 1, t, out_t)
                else:
                    ohp = wk.tile([P, P], BF16, tag="ohp")
                    nc.vector.tensor_tensor(
                        out=ohp[:],
                        in0=bsh_t[:, t:t + 1].to_broadcast([P, P]),
                        in1=iota_f[:], op=mybir.AluOpType.is_equal)
                    nc.tensor.matmul(pool_ps[:], lhsT=ohp[:], rhs=out_t[:],
                                     start=(t == 0), stop=(t == NT - 1))

            def edge_pass_lo(l):
                for t in range(NT):
                    acc = psA.tile([P, DA], F32, space="PSUM", tag="acc")
                    self_loop(t, acc)
                    edge_groups(t, 0, acc, start=False)
                    nc.vector.tensor_copy(part_sb[:, t * DA:(t + 1) * DA],
                                          acc[:])

            def edge_pass_hi(l, pool_ps):
                for t in range(NT):
                    if m_hi[t] > 0:
                        acc = psA.tile([P, DA], F32, space="PSUM", tag="acc")
                        edge_groups(t, 1, acc, start=True)
                        sum_t = wk.tile([P, DA], F32, tag="sum")
                        nc.vector.tensor_add(sum_t[:],
                                             part_sb[:, t * DA:(t + 1) * DA],
                                             acc[:])
                    else:
                        sum_t = wk.tile([P, DA], F32, tag="sum")
                        nc.vector.tensor_copy(
                            sum_t[:], part_sb[:, t * DA:(t + 1) * DA])
                    epilogue(l, t, sum_t, pool_ps)
                    if l < 2:
                        if t == TA - 1:
                            all_gather(0)
                        elif t == NT - 1:
                            all_gather(1)

            # ---------------- run
            for _rep in range(reps):
                # layer-0 node sweep over x
                for t in range(NT):
                    in_t = wk.tile([P, IN_CH], BF16, tag="x_t")
                    nc.sync.dma_start(in_t[:], x_in[t * P:(t + 1) * P, :])
                    node_tile(0, t, in_t)
                    if t == TA - 1:
                        all_gather(0)
                all_gather(1)

                for l in range(3):
                    pool_ps = None
                    if l == 2:
                        pool_ps = psP.tile([P, DH], F32, space="PSUM",
                                           tag="pool")
                    if dup == "lo":
                        edge_pass_lo(l)
                    edge_pass_lo(l)
                    edge_pass_hi(l, pool_ps)

                # scatter pool partials and AllReduce
                zt = wk.tile([P, DH], F32, tag="zero")
                nc.gpsimd.memset(zt[:], 0.0)
                for b in range(GP // P):
                    nc.sync.dma_start(pool_loc[b * P:(b + 1) * P, :], zt[:])
                pl = wk.tile([P, DH], F32, tag="plocal")
                nc.vector.tensor_copy(pl[:], pool_ps[:])
                nc.gpsimd.indirect_dma_start(
                    out=pool_loc[:, :],
                    out_offset=bass.IndirectOffsetOnAxis(
                        ap=gslot_t[:, 0:1], axis=0),
                    in_=pl[:, :], in_offset=None,
                    bounds_check=GP - 1, oob_is_err=False)
                if no_cc:
                    nc.sync.dma_start(pool_sum[:, :], pool_loc[:, :])
                else:
                    nc.gpsimd.collective_compute(
                        "AllReduce", mybir.AluOpType.add,
                        ins=[pool_loc[:, :].opt()],
                        outs=[pool_sum[:, :].opt()],
                        replica_groups=[list(range(ncores))])

            # final linear: y = (pool_sum * invcnt) @ lin_w + lin_b
            for b in range(GP // P):
                pt = wk.tile([P, DH], F32, tag="psum_t")
                nc.sync.dma_start(pt[:], pool_sum[b * P:(b + 1) * P, :])
                ic = wk.tile([P, 1], F32, tag="ic")
                nc.sync.dma_start(ic[:], invcnt[b * P:(b + 1) * P, :])
                mulw = wk.tile([P, DH], F32, tag="mulw")
                nc.vector.tensor_mul(mulw[:], pt[:], linw_t[:])
                rs = wk.tile([P, 1], F32, tag="rs")
                nc.vector.reduce_sum(rs[:], mulw[:], axis=mybir.AxisListType.X)
                nc.vector.tensor_mul(rs[:], rs[:], ic[:])
                nc.vector.tensor_add(rs[:], rs[:], linb_t[:])
                nc.sync.dma_start(y[b * P:(b + 1) * P, :], rs[:])

    nc.compile()
    return nc


# ------------------------------------------------------------------- runner

class SpmdRunner:
    def __init__(self, nc, n_cores):
        import jax
        from jax.sharding import Mesh, PartitionSpec
        from jax.experimental.shard_map import shard_map
        from concourse.bass2jax import (
            _bass_exec_p, install_neuronx_cc_hook, partition_id_tensor)
        self.jax = jax
        install_neuronx_cc_hook()
        self.nc = nc
        self.n_cores = n_cores
        partition_name = (nc.partition_id_tensor.name
                          if nc.partition_id_tensor else None)
        in_names, out_names, out_avals, zero_outs = [], [], [], []
        for alloc in nc.m.functions[0].allocations:
            if not isinstance(alloc, mybir.MemoryLocationSet):
                continue
            name = alloc.memorylocations[0].name
            if alloc.kind == "ExternalInput":
                if name != partition_name and name != (
                        nc.dbg_addr.name if nc.dbg_addr else None):
                    in_names.append(name)
            elif alloc.kind == "ExternalOutput":
                out_names.append(name)
                shape = tuple(alloc.tensor_shape)
                dtype = mybir.dt.np(alloc.dtype)
                out_avals.append(jax.core.ShapedArray(shape, dtype))
                zero_outs.append(np.zeros(shape, dtype))
        self.in_names, self.out_names = in_names, out_names
        self.out_avals, self.zero_outs = out_avals, zero_outs
        n_params = len(in_names)
        all_in_names = list(in_names) + list(out_names)
        has_dbg = nc.dbg_addr is not None
        if has_dbg:
            all_in_names.append(nc.dbg_addr.name)
        if partition_name is not None:
            all_in_names.append(partition_name)

        def _body(*args):
            operands = list(args)
            if has_dbg:
                operands.append(jax.numpy.zeros((1, 2), jax.numpy.uint32))
            if partition_name is not None:
                operands.append(partition_id_tensor())
            outs = _bass_exec_p.bind(
                *operands, out_avals=tuple(out_avals),
                in_names=tuple(all_in_names), out_names=tuple(out_names),
                lowering_input_output_aliases=(),
                sim_require_finite=False, sim_require_nnan=False, nc=nc)
            return tuple(outs)

        devices = jax.devices()[:n_cores]
        assert len(devices) == n_cores
        mesh = Mesh(np.asarray(devices), ("core",))
        in_specs = (PartitionSpec("core"),) * (n_params + len(out_names))
        out_specs = (PartitionSpec("core"),) * len(out_names)
        self.fn = jax.jit(
            shard_map(_body, mesh=mesh, in_specs=in_specs,
                      out_specs=out_specs, check_rep=False),
            keep_unused=True)

    def prepare(self, in_maps):
        per_core = [[np.ascontiguousarray(m[nm]) for nm in self.in_names]
                    for m in in_maps]
        concat_in = [
            np.concatenate([per_core[c][i] for c in range(self.n_cores)],
                           axis=0)
            for i in range(len(self.in_names))]
        concat_zero = [
            np.zeros((self.n_cores * z.shape[0], *z.shape[1:]), z.dtype)
            for z in self.zero_outs]
        args = [self.jax.device_put(a) for a in concat_in + concat_zero]
        for a in args:
            a.block_until_ready()
        return args

    def run(self, args):
        outs = self.fn(*args)
        self.jax.block_until_ready(outs)
        return outs

    def results(self, outs):
        res = []
        for c in range(self.n_cores):
            m = {}
            for i, nm in enumerate(self.out_names):
                m[nm] = np.asarray(outs[i]).reshape(
                    self.n_cores, *self.out_avals[i].shape)[c]
            res.append(m)
        return res


# -------------------------------------------------------------------- kernel

_CACHE = {}

N_FULL, E_FULL, G_FULL, NCORES = 50000, 800000, 512, 8


def kernel(x, edge_index, batch,
           W0, a_src0, a_dst0, bias0,
           W1, a_src1, a_dst1, bias1,
           W2, a_src2, a_dst2, bias2,
           lin_w, lin_b):
    x = np.asarray(x, np.float32)
    edge_index = np.asarray(edge_index, np.int64)
    batch = np.asarray(batch, np.int64)
    N, E, G = x.shape[0], edge_index.shape[1], G_FULL

    cfg, in_maps = host_prep(
        x, edge_index, batch,
        [np.asarray(W0, np.float32), np.asarray(W1, np.float32),
         np.asarray(W2, np.float32)],
        [np.asarray(a_src0, np.float32), np.asarray(a_src1, np.float32),
         np.asarray(a_src2, np.float32)],
        [np.asarray(a_dst0, np.float32), np.asarray(a_dst1, np.float32),
         np.asarray(a_dst2, np.float32)],
        [np.asarray(bias0, np.float32), np.asarray(bias1, np.float32),
         np.asarray(bias2, np.float32)],
        np.asarray(lin_w, np.float32), np.asarray(lin_b, np.float32),
        N, E, G, NCORES)

    key = (cfg["NL"], cfg["NT"], cfg["GP"], cfg["m_lo"], cfg["m_hi"],
           cfg["SM"], cfg["ncores"])
    if key not in _CACHE:
        nc = build_program(cfg)
        _CACHE[key] = (nc, SpmdRunner(nc, NCORES))
    nc, runner = _CACHE[key]

    args = runner.prepare(in_maps)
    outs = runner.run(args)
    res = runner.results(outs)
    return res[0]["y"][:G].astype(np.float32)


# revision 10
# speedup vs baseline: 2.3972x; 1.0150x over previous
"""3-layer GAT (8 heads x 32 hid, PyG GATConv semantics w/ self-loops) +
global mean pool + linear, distributed over 8 Trainium2 NeuronCores.

v2 strategy: nodes partitioned into 8 contiguous ranges (dst-owner);
non-self-loop edges live with their dst core, bucketed per (dst tile,
src chunk), sorted by src. All node features flow through a bf16 table
hh = [h | alpha_src] (DG=384-wide rows, 768B — dma_gather elem granularity).

Per layer:
  - node transform is FUSED into the previous layer's edge epilogue
    (layer 0 has a standalone node sweep over x).
  - hh is AllGathered in TWO chunks (tiles 0-24 -> hh_A, tiles 25-48 ->
    hh_B) fired as soon as the respective node tiles finish, so the
    collectives overlap edge-phase compute.
  - edge phase runs two passes: pass-lo processes edges whose src is in
    chunk A (overlapping the in-flight AG of chunk B), drains partial
    [num|den] per dst tile to SBUF; pass-hi adds chunk-B edges, then the
    epilogue computes out = elu(num/den + bias) and immediately runs the
    next layer's node matmul for that tile.
  - self-loop contributions are computed locally (no gather) and seed the
    PSUM accumulator, which also guarantees den>0 for padded rows.
  - segment softmax is algebraically folded: out = sum(ex*h)/sum(ex),
    no max subtraction (attention logits are O(1)-bounded).
  - gathers use dma_gather with 512 rows/instr, alternating SWDGE queues
    (descriptor generation parallelizes across queues).
Per-graph mean-pool partials are scattered into a 512-row buffer and
AllReduced; the final linear layer is tiny.

Self-contained: hardcodes the problem shapes; host-side preprocessing uses
only graph structure (edge_index, batch) and parameter repacking.
"""
import numpy as np
import ml_dtypes

import concourse.bass as bass
import concourse.bacc as bacc
import concourse.mybir as mybir
import concourse.tile as tile

P = 128
KB = 4                    # 128-edge groups per dma_gather
NQ = 4                    # SWDGE queues for gather descriptor generation
WORK_BUFS = 8
GATHER_BUFS = 4
HEADS, HID = 8, 32
DH = HEADS * HID          # 256
DA = DH + HEADS           # 264 = h | alpha_src
DW = DH + 2 * HEADS       # 272 = W | W@Asrc | W@Adst
DG = 384                  # bf16 row padded so DG*2B % 256B == 0
IN_CH = 128
NEG = 0.2
F32 = mybir.dt.float32
I32 = mybir.dt.int32
BF16 = mybir.dt.bfloat16
EXP = mybir.ActivationFunctionType.Exp


# ----------------------------------------------------------------- host prep

def host_prep(x, edge_index, batch, Ws, a_srcs, a_dsts, biases, lin_w, lin_b,
              N, E, G, ncores):
    """Build per-core input maps + the (core-uniform) program config."""
    nl = N // ncores
    assert nl * ncores == N
    NL = ((nl + P - 1) // P) * P               # padded local nodes (6272)
    NT = NL // P                               # dst tiles per core (49)
    TA = (NT + 1) // 2                         # chunk-A tiles (25)
    CHA, CHB = TA * P, (NT - TA) * P           # 3200 / 3072 local rows
    GP = ((G + P - 1) // P) * P
    assert 8 * CHA <= 32768 and 8 * CHB <= 32768

    src = np.asarray(edge_index[0])
    dst = np.asarray(edge_index[1])
    core_of = dst // nl
    dloc_all = dst - core_of * nl

    s_core = src // nl
    s_loc = src - s_core * nl
    s_half = (s_loc >= CHA).astype(np.int64)
    gsrc = np.where(s_half == 0, s_core * CHA + s_loc,
                    s_core * CHB + (s_loc - CHA))

    # per-core, per-tile, per-half edge lists sorted by gsrc
    per_core = []
    cnt = np.zeros((ncores, NT, 2), np.int64)
    for k in range(ncores):
        m = core_of == k
        g_k, d_k, h_k = gsrc[m], dloc_all[m], s_half[m]
        order = np.lexsort((g_k, h_k, d_k // P))
        g_k, d_k, h_k = g_k[order], d_k[order], h_k[order]
        t_k = d_k // P
        tiles = []
        for t in range(NT):
            mt = t_k == t
            g_t, d_t, h_t = g_k[mt], d_k[mt] - t * P, h_k[mt]
            lo = (g_t[h_t == 0], d_t[h_t == 0])
            hi = (g_t[h_t == 1], d_t[h_t == 1])
            tiles.append((lo, hi))
            cnt[k, t, 0] = len(lo[0])
            cnt[k, t, 1] = len(hi[0])
        per_core.append(tiles)
    cmax = cnt.max(axis=0)                                  # [NT, 2]
    m_lo = [int(v) for v in np.ceil(cmax[:, 0] / P).astype(np.int64)]
    m_hi = [int(v) for v in np.ceil(cmax[:, 1] / P).astype(np.int64)]
    SM = int(sum(m_lo) + sum(m_hi))
    # column offset of each tile's run (all-lo first, then all-hi)
    col_lo, col_hi = [], []
    c = 0
    for t in range(NT):
        col_lo.append(c)
        c += m_lo[t]
    for t in range(NT):
        col_hi.append(c)
        c += m_hi[t]

    in_maps = []
    batch = np.asarray(batch, np.int64)
    for k in range(ncores):
        dstflat = np.full((P, SM), 999.0, np.float32)
        idx16 = np.zeros((16, 8 * SM), np.int16)
        for t in range(NT):
            for half, mh, c0 in ((0, m_lo[t], col_lo[t]),
                                 (1, m_hi[t], col_hi[t])):
                if mh == 0:
                    continue
                g_t, d_t = per_core[k][t][half]
                L = mh * P
                gi = np.zeros(L, np.int64)
                gi[:len(g_t)] = g_t
                dd = np.full(L, 999.0, np.float32)
                dd[:len(d_t)] = d_t
                ii = np.arange(L)
                dstflat[ii % P, c0 + ii // P] = dd
                j = 0
                while j < mh:
                    kb = min(KB, mh - j)
                    arr = gi[j * P:(j + kb) * P]
                    idx16[:, (c0 + j) * 8:(c0 + j + kb) * 8] = \
                        arr.reshape(-1, 16).T.astype(np.int16)
                    j += kb

        bl = batch[k * nl:(k + 1) * nl]
        g_base = int(bl[0])
        bshift = np.full((NL,), 999.0, np.float32)
        bshift[:nl] = (bl - g_base).astype(np.float32)
        gslot = np.arange(g_base, g_base + P, dtype=np.int64)
        gslot = np.where(gslot < G, gslot, 99999).astype(np.int32)[:, None]

        xk = np.zeros((NL, IN_CH), np.float32)
        xk[:nl] = x[k * nl:(k + 1) * nl]

        # precomputed one-hot blocks: per (tile, half) run, oh (e->d) then
        # ohT (d->e), each group 128 cols, packed in global group order
        ohx = np.zeros((P, 2 * SM * P), ml_dtypes.bfloat16)
        dvals = dstflat  # [P(e), SM], 999 for pads
        lane = np.arange(P, dtype=np.float32)
        for t in range(NT):
            for mh, c0 in ((m_lo[t], col_lo[t]), (m_hi[t], col_hi[t])):
                if mh == 0:
                    continue
                dd = dvals[:, c0:c0 + mh]                     # [e, mh]
                oh = (dd[:, :, None] == lane[None, None, :])  # [e, mh, d]
                base = 2 * c0 * P
                ohx[:, base:base + mh * P] = \
                    oh.reshape(P, mh * P).astype(ml_dtypes.bfloat16)
                ohT = np.transpose(oh, (2, 1, 0))             # [d, mh, e]
                ohx[:, base + mh * P:base + 2 * mh * P] = \
                    ohT.reshape(P, mh * P).astype(ml_dtypes.bfloat16)

        im = {
            "x_local": xk.astype(ml_dtypes.bfloat16),
            "idx16": np.tile(idx16, (8, 1)),
            "ohx": ohx,
            "bshift": bshift.reshape(NT, P).T.copy(),     # [P, NT]
            "gslot": gslot,
        }
        in_maps.append(im)

    # replicated parameters
    def aug(W, a_s, a_d):
        As = np.zeros((DH, HEADS), np.float64)
        Ad = np.zeros((DH, HEADS), np.float64)
        for h in range(HEADS):
            As[h * HID:(h + 1) * HID, h] = a_s[h]
            Ad[h * HID:(h + 1) * HID, h] = a_d[h]
        W = W.astype(np.float64)
        out = np.concatenate([W, W @ As, W @ Ad], axis=1)
        return out.astype(ml_dtypes.bfloat16)

    cnts = np.bincount(batch, minlength=G).astype(np.float32)
    invcnt = np.zeros((GP, 1), np.float32)
    invcnt[:G, 0] = 1.0 / np.maximum(cnts, 1.0)
    params = {
        "W0aug": aug(Ws[0], a_srcs[0], a_dsts[0]),
        "W1aug": aug(Ws[1], a_srcs[1], a_dsts[1]),
        "W2aug": aug(Ws[2], a_srcs[2], a_dsts[2]),
        "b0": np.tile(biases[0][None, :], (P, 1)).astype(np.float32),
        "b1": np.tile(biases[1][None, :], (P, 1)).astype(np.float32),
        "b2": np.tile(biases[2][None, :], (P, 1)).astype(np.float32),
        "linw": np.tile(lin_w.reshape(1, DH), (P, 1)).astype(np.float32),
        "linb": np.tile(lin_b.reshape(1, 1), (P, 1)).astype(np.float32),
        "invcnt": invcnt,
    }
    for im in in_maps:
        im.update(params)

    cfg = dict(NL=NL, NT=NT, TA=TA, GP=GP, m_lo=tuple(m_lo), m_hi=tuple(m_hi),
               col_lo=tuple(col_lo), col_hi=tuple(col_hi), SM=SM,
               ncores=ncores)
    return cfg, in_maps


# ------------------------------------------------------------- program build

def build_program(cfg, reps=1, dup=None, no_cc=False):
    NL, NT, TA, GP = cfg["NL"], cfg["NT"], cfg["TA"], cfg["GP"]
    SM, ncores = cfg["SM"], cfg["ncores"]
    m_lo, m_hi = cfg["m_lo"], cfg["m_hi"]
    col_lo, col_hi = cfg["col_lo"], cfg["col_hi"]
    CHA, CHB = TA * P, (NT - TA) * P

    nc = bacc.Bacc("TRN2", target_bir_lowering=False, debug=False,
                   num_devices=ncores, dynamic_dma_scratch_size=32768,
                   num_swdge_queues=NQ)
    # ---------------- I/O
    x_in = nc.dram_tensor("x_local", [NL, IN_CH], BF16, kind="ExternalInput")
    idx16 = nc.dram_tensor("idx16", [P, 8 * SM], mybir.dt.int16,
                           kind="ExternalInput")
    ohx = nc.dram_tensor("ohx", [P, 2 * SM * P], BF16, kind="ExternalInput")
    bshift = nc.dram_tensor("bshift", [P, NT], F32, kind="ExternalInput")
    gslot = nc.dram_tensor("gslot", [P, 1], I32, kind="ExternalInput")
    Waug = [nc.dram_tensor(f"W{l}aug", [IN_CH if l == 0 else DH, DW], BF16,
                           kind="ExternalInput") for l in range(3)]
    bias = [nc.dram_tensor(f"b{l}", [P, DH], F32, kind="ExternalInput")
            for l in range(3)]
    linw = nc.dram_tensor("linw", [P, DH], F32, kind="ExternalInput")
    linb = nc.dram_tensor("linb", [P, 1], F32, kind="ExternalInput")
    invcnt = nc.dram_tensor("invcnt", [GP, 1], F32, kind="ExternalInput")
    y = nc.dram_tensor("y", [GP, 1], F32, kind="ExternalOutput")

    with tile.TileContext(nc) as tc:
        with tc.tile_pool(name="const", bufs=1) as cst, \
             tc.tile_pool(name="dram", bufs=1, space="DRAM") as dram, \
             tc.tile_pool(name="work", bufs=WORK_BUFS) as wk, \
             tc.tile_pool(name="gpool", bufs=GATHER_BUFS) as gp, \
             tc.tile_pool(name="ohpool", bufs=3) as ohp_pool, \
             tc.tile_pool(name="psA", bufs=2, space="PSUM") as psA, \
             tc.tile_pool(name="psB", bufs=2, space="PSUM") as psB, \
             tc.tile_pool(name="psC", bufs=1, space="PSUM") as psC, \
             tc.tile_pool(name="psN", bufs=1, space="PSUM") as psN, \
             tc.tile_pool(name="psP", bufs=1, space="PSUM") as psP:

            # ---------------- DRAM intermediates
            hh_local = dram.tile([NL, DG], BF16)
            hh_A = dram.tile([ncores * CHA, DG], BF16)
            hh_B = dram.tile([ncores * CHB, DG], BF16)
            pool_loc = dram.tile([GP, DH], F32)
            pool_sum = dram.tile([GP, DH], F32)

            # ---------------- constants
            iota_i = cst.tile([P, P], I32)
            nc.gpsimd.iota(iota_i[:], pattern=[[1, P]], base=0,
                           channel_multiplier=0)
            iota_b = cst.tile([P, P], BF16)
            nc.vector.tensor_copy(iota_b[:], iota_i[:])
            iota_f = cst.tile([P, P], F32)
            nc.vector.tensor_copy(iota_f[:], iota_i[:])
            iota_ci = cst.tile([P, 1], I32)
            nc.gpsimd.iota(iota_ci[:], pattern=[[0, 1]], base=0,
                           channel_multiplier=1)
            iota_cf = cst.tile([P, 1], F32)
            nc.vector.tensor_copy(iota_cf[:], iota_ci[:])
            ident_b = cst.tile([P, P], BF16)
            nc.vector.tensor_tensor(out=ident_b[:],
                                    in0=iota_cf[:].to_broadcast([P, P]),
                                    in1=iota_f[:], op=mybir.AluOpType.is_equal)

            idx_all = cst.tile([P, 8 * SM], mybir.dt.int16)
            nc.sync.dma_start(idx_all[:], idx16[:, :])
            bsh_t = cst.tile([P, NT], F32)
            nc.sync.dma_start(bsh_t[:], bshift[:, :])

            W_t = []
            for l in range(3):
                cin = IN_CH if l == 0 else DH
                tiles = []
                for kk in range(cin // P):
                    t = cst.tile([P, DW], BF16, tag=f"W{l}_{kk}")
                    nc.sync.dma_start(t[:], Waug[l][kk * P:(kk + 1) * P, :])
                    tiles.append(t)
                W_t.append(tiles)
            bias_t = []
            for l in range(3):
                t = cst.tile([P, DH], F32, tag=f"bias{l}")
                nc.sync.dma_start(t[:], bias[l][:, :])
                bias_t.append(t)
            linw_t = cst.tile([P, DH], F32)
            nc.sync.dma_start(linw_t[:], linw[:, :])
            linb_t = cst.tile([P, 1], F32)
            nc.sync.dma_start(linb_t[:], linb[:, :])
            gslot_t = cst.tile([P, 1], I32)
            nc.sync.dma_start(gslot_t[:], gslot[:, :])

            # zero hh_local pad columns once (they ride along in the AG)
            zpad = cst.tile([P, DG - DA], BF16)
            nc.gpsimd.memset(zpad[:], 0.0)
            for nt in range(NT):
                nc.sync.dma_start(hh_local[nt * P:(nt + 1) * P, DA:DG],
                                  zpad[:])

            # per-layer alpha_src/alpha_dst for local nodes, kept in SBUF
            as_sb = cst.tile([P, NT * HEADS], F32)
            ad_sb = cst.tile([P, NT * HEADS], F32)
            # pass-lo partial [num|den] per dst tile
            part_sb = cst.tile([P, NT * DA], F32)

            gq = [0]  # gather queue round-robin counter

            # ---------------- helpers
            def node_tile(l, t, src_bf):
                """src_bf [P, cin] bf16 -> hh_local[t], as_sb/ad_sb col t."""
                cin = IN_CH if l == 0 else DH
                ps_o = psN.tile([P, DW], F32, space="PSUM", tag="node_mm")
                for kk in range(cin // P):
                    trp = psC.tile([P, P], BF16, space="PSUM", tag="trp")
                    nc.tensor.transpose(out=trp[:],
                                        in_=src_bf[:, kk * P:(kk + 1) * P],
                                        identity=ident_b[:])
                    inT = wk.tile([P, P], BF16, tag="node_inT")
                    nc.vector.tensor_copy(inT[:], trp[:])
                    nc.tensor.matmul(ps_o[:], lhsT=inT[:], rhs=W_t[l][kk][:],
                                     start=(kk == 0), stop=(kk == cin // P - 1))
                hh_t = wk.tile([P, DA], BF16, tag="node_hh")
                nc.vector.tensor_copy(hh_t[:], ps_o[:, 0:DA])
                nc.sync.dma_start(hh_local[t * P:(t + 1) * P, 0:DA], hh_t[:])
                nc.vector.tensor_copy(as_sb[:, t * HEADS:(t + 1) * HEADS],
                                      ps_o[:, DH:DH + HEADS])
                nc.vector.tensor_copy(ad_sb[:, t * HEADS:(t + 1) * HEADS],
                                      ps_o[:, DH + HEADS:DW])

            def all_gather(chunk):
                if no_cc:
                    if chunk == 0:
                        nc.sync.dma_start(hh_A[0:CHA, :], hh_local[0:CHA, :])
                    else:
                        nc.sync.dma_start(hh_B[0:CHB, :],
                                          hh_local[CHA:NL, :])
                    return
                if chunk == 0:
                    nc.gpsimd.collective_compute(
                        "AllGather", mybir.AluOpType.bypass,
                        ins=[hh_local[0:CHA, :].opt()],
                        outs=[hh_A[:, :].opt()],
                        replica_groups=[list(range(ncores))])
                else:
                    nc.gpsimd.collective_compute(
                        "AllGather", mybir.AluOpType.bypass,
                        ins=[hh_local[CHA:NL, :].opt()],
                        outs=[hh_B[:, :].opt()],
                        replica_groups=[list(range(ncores))])

            def edge_groups(t, half, acc, start):
                """Process the gather groups of (tile t, half). Returns True
                if any matmul was issued (acc live)."""
                mh = (m_lo if half == 0 else m_hi)[t]
                if mh == 0:
                    return False
                c0 = (col_lo if half == 0 else col_hi)[t]
                src_d = hh_A if half == 0 else hh_B
                ad_t = wk.tile([P, HEADS], BF16, tag="ad")
                nc.vector.tensor_copy(ad_t[:],
                                      ad_sb[:, t * HEADS:(t + 1) * HEADS])
                oh_t = ohp_pool.tile([P, 2 * mh * P], BF16, tag="oht")
                nc.sync.dma_start(
                    oh_t[:], ohx[:, 2 * c0 * P:2 * (c0 + mh) * P])
                j = 0
                while j < mh:
                    kb = min(KB, mh - j)
                    co = c0 + j
                    g4 = gp.tile([P, KB * DG], BF16, tag="hhg")
                    adg4 = psB.tile([P, KB * HEADS], F32, space="PSUM",
                                    tag="adg")
                    nc.gpsimd.dma_gather(
                        out_ap=g4[:, 0:kb * DG].rearrange(
                            "p (q d) -> p q d", q=kb),
                        in_ap=src_d[:, :],
                        idxs_ap=idx_all[:, co * 8:(co + kb) * 8],
                        num_idxs=kb * P, num_idxs_reg=kb * P,
                        elem_size=DG, queue_num=gq[0] % NQ)
                    gq[0] += 1
                    ohs = []
                    for q in range(kb):
                        oh = oh_t[:, (j + q) * P:(j + q + 1) * P]
                        ohs.append(oh)
                        ohT = oh_t[:, (mh + j + q) * P:(mh + j + q + 1) * P]
                        nc.tensor.matmul(
                            adg4[:, q * HEADS:(q + 1) * HEADS], lhsT=ohT,
                            rhs=ad_t[:], start=True, stop=True)
                    rhs4 = gp.tile([P, KB * DA], BF16, tag="rhs")
                    e4 = wk.tile([P, KB * HEADS], F32, tag="e")
                    nc.vector.tensor_add(
                        e4[:, 0:kb * HEADS].rearrange(
                            "p (q h) -> p q h", q=kb),
                        g4[:, 0:kb * DG].rearrange(
                            "p (q d) -> p q d", q=kb)[:, :, DH:DA],
                        adg4[:, 0:kb * HEADS].rearrange(
                            "p (q h) -> p q h", q=kb))
                    nc.vector.scalar_tensor_tensor(
                        out=e4[:, 0:kb * HEADS], in0=e4[:, 0:kb * HEADS],
                        scalar=NEG, in1=e4[:, 0:kb * HEADS],
                        op0=mybir.AluOpType.mult, op1=mybir.AluOpType.max)
                    nc.scalar.activation(
                        rhs4[:, 0:kb * DA].rearrange(
                            "p (q d) -> p q d", q=kb)[:, :, DH:DA],
                        e4[:, 0:kb * HEADS].rearrange(
                            "p (q h) -> p q h", q=kb), EXP)
                    nc.vector.tensor_mul(
                        rhs4[:, 0:kb * DA].rearrange(
                            "p (q d) -> p q d", q=kb)[:, :, 0:DH].rearrange(
                            "p q (h c) -> p q h c", h=HEADS),
                        g4[:, 0:kb * DG].rearrange(
                            "p (q d) -> p q d", q=kb)[:, :, 0:DH].rearrange(
                            "p q (h c) -> p q h c", h=HEADS),
                        rhs4[:, 0:kb * DA].rearrange(
                            "p (q d) -> p q d", q=kb)[:, :, DH:DA][
                            :, :, :, None].to_broadcast(
                            [P, kb, HEADS, HID]))
                    for q in range(kb):
                        nc.tensor.matmul(
                            acc[:], lhsT=ohs[q],
                            rhs=rhs4[:, q * DA:(q + 1) * DA],
                            start=(start and j == 0 and q == 0),
                            stop=(j + kb >= mh and q == kb - 1))
                    j += kb
                return mh > 0

            def self_loop(t, acc):
                """Seed acc with the self-loop term (start=True matmul)."""
                hsrc = wk.tile([P, DH], BF16, tag="hself")
                nc.sync.dma_start(hsrc[:], hh_local[t * P:(t + 1) * P, 0:DH])
                es = wk.tile([P, HEADS], F32, tag="eself")
                nc.vector.tensor_add(es[:],
                                     as_sb[:, t * HEADS:(t + 1) * HEADS],
                                     ad_sb[:, t * HEADS:(t + 1) * HEADS])
                nc.vector.scalar_tensor_tensor(
                    out=es[:], in0=es[:], scalar=NEG, in1=es[:],
                    op0=mybir.AluOpType.mult, op1=mybir.AluOpType.max)
                rhs_s = wk.tile([P, DA], BF16, tag="rhs_s")
                nc.scalar.activation(rhs_s[:, DH:DA], es[:], EXP)
                nc.vector.tensor_mul(
                    rhs_s[:, 0:DH].rearrange("p (h c) -> p h c", h=HEADS),
                    hsrc[:].rearrange("p (h c) -> p h c", h=HEADS),
                    rhs_s[:, DH:DA][:, :, None].to_broadcast([P, HEADS, HID]))
                nc.tensor.matmul(acc[:], lhsT=ident_b[:], rhs=rhs_s[:],
                                 start=True, stop=(m_lo[t] == 0))

            def epilogue(l, t, sum_t, pool_ps):
                """sum_t [P, DA] f32 -> out bf16; fused next-layer node mm."""
                inv_t = wk.tile([P, HEADS], F32, tag="inv")
                nc.vector.reciprocal(inv_t[:], sum_t[:, DH:DA])
                h0 = wk.tile([P, DH], F32, tag="h0")
                nc.vector.tensor_mul(
                    h0[:].rearrange("p (h c) -> p h c", h=HEADS),
                    sum_t[:, 0:DH].rearrange("p (h c) -> p h c", h=HEADS),
                    inv_t[:, :, None].to_broadcast([P, HEADS, HID]))
                nc.vector.tensor_add(h0[:], h0[:], bias_t[l][:])
                tm = wk.tile([P, DH], F32, tag="tm")
                nc.vector.tensor_scalar_min(tm[:], h0[:], 0.0)
                nc.scalar.activation(tm[:], tm[:], EXP)
                out_t = wk.tile([P, DH], BF16, tag="hout")
                nc.vector.scalar_tensor_tensor(
                    out=out_t[:], in0=h0[:], scalar=0.0, in1=tm[:],
                    op0=mybir.AluOpType.max, op1=mybir.AluOpType.add)
                nc.vector.tensor_scalar_add(out_t[:], out_t[:], -1.0)
                if l < 2:
                    node_tile(l + 1, t, out_t)
                else:
                    ohp = wk.tile([P, P], BF16, tag="ohp")
                    nc.vector.tensor_tensor(
                        out=ohp[:],
                        in0=bsh_t[:, t:t + 1].to_broadcast([P, P]),
                        in1=iota_f[:], op=mybir.AluOpType.is_equal)
                    nc.tensor.matmul(pool_ps[:], lhsT=ohp[:], rhs=out_t[:],
                                     start=(t == 0), stop=(t == NT - 1))

            def edge_pass_lo(l):
                for t in range(NT):
                    acc = psA.tile([P, DA], F32, space="PSUM", tag="acc")
                    self_loop(t, acc)
                    edge_groups(t, 0, acc, start=False)
                    nc.vector.tensor_copy(part_sb[:, t * DA:(t + 1) * DA],
                                          acc[:])

            def edge_pass_hi(l, pool_ps):
                for t in range(NT):
                    if m_hi[t] > 0:
                        acc = psA.tile([P, DA], F32, space="PSUM", tag="acc")
                        edge_groups(t, 1, acc, start=True)
                        sum_t = wk.tile([P, DA], F32, tag="sum")
                        nc.vector.tensor_add(sum_t[:],
                                             part_sb[:, t * DA:(t + 1) * DA],
                                             acc[:])
                    else:
                        sum_t = wk.tile([P, DA], F32, tag="sum")
                        nc.vector.tensor_copy(
                            sum_t[:], part_sb[:, t * DA:(t + 1) * DA])
                    epilogue(l, t, sum_t, pool_ps)
                    if l < 2:
                        if t == TA - 1:
                            all_gather(0)
                        elif t == NT - 1:
                            all_gather(1)

            # ---------------- run
            for _rep in range(reps):
                # layer-0 node sweep over x
                for t in range(NT):
                    in_t = wk.tile([P, IN_CH], BF16, tag="x_t")
                    nc.sync.dma_start(in_t[:], x_in[t * P:(t + 1) * P, :])
                    node_tile(0, t, in_t)
                    if t == TA - 1:
                        all_gather(0)
                all_gather(1)

                for l in range(3):
                    pool_ps = None
                    if l == 2:
                        pool_ps = psP.tile([P, DH], F32, space="PSUM",
                                           tag="pool")
                    if dup == "lo":
                        edge_pass_lo(l)
                    edge_pass_lo(l)
                    edge_pass_hi(l, pool_ps)

                # scatter pool partials and AllReduce
                zt = wk.tile([P, DH], F32, tag="zero")
                nc.gpsimd.memset(zt[:], 0.0)
                for b in range(GP // P):
                    nc.sync.dma_start(pool_loc[b * P:(b + 1) * P, :], zt[:])
                pl = wk.tile([P, DH], F32, tag="plocal")
                nc.vector.tensor_copy(pl[:], pool_ps[:])
                nc.gpsimd.indirect_dma_start(
                    out=pool_loc[:, :],
                    out_offset=bass.IndirectOffsetOnAxis(
                        ap=gslot_t[:, 0:1], axis=0),
                    in_=pl[:, :], in_offset=None,
                    bounds_check=GP - 1, oob_is_err=False)
                if no_cc:
                    nc.sync.dma_start(pool_sum[:, :], pool_loc[:, :])
                else:
                    nc.gpsimd.collective_compute(
                        "AllReduce", mybir.AluOpType.add,
                        ins=[pool_loc[:, :].opt()],
                        outs=[pool_sum[:, :].opt()],
                        replica_groups=[list(range(ncores))])

            # final linear: y = (pool_sum * invcnt) @ lin_w + lin_b
            for b in range(GP // P):
                pt = wk.tile([P, DH], F32, tag="psum_t")
                nc.sync.dma_start(pt[:], pool_sum[b * P:(b + 1) * P, :])
                ic = wk.tile([P, 1], F32, tag="ic")
                nc.sync.dma_start(ic[:], invcnt[b * P:(b + 1) * P, :])
                mulw = wk.tile([P, DH], F32, tag="mulw")
                nc.vector.tensor_mul(mulw[:], pt[:], linw_t[:])
                rs = wk.tile([P, 1], F32, tag="rs")
                nc.vector.reduce_sum(rs[:], mulw[:], axis=mybir.AxisListType.X)
                nc.vector.tensor_mul(rs[:], rs[:], ic[:])
                nc.vector.tensor_add(rs[:], rs[:], linb_t[:])
                nc.sync.dma_start(y[b * P:(b + 1) * P, :], rs[:])

    nc.compile()
    return nc


# ------------------------------------------------------------------- runner

class SpmdRunner:
    def __init__(self, nc, n_cores):
        import jax
        from jax.sharding import Mesh, PartitionSpec
        from jax.experimental.shard_map import shard_map
        from concourse.bass2jax import (
            _bass_exec_p, install_neuronx_cc_hook, partition_id_tensor)
        self.jax = jax
        install_neuronx_cc_hook()
        self.nc = nc
        self.n_cores = n_cores
        partition_name = (nc.partition_id_tensor.name
                          if nc.partition_id_tensor else None)
        in_names, out_names, out_avals, zero_outs = [], [], [], []
        for alloc in nc.m.functions[0].allocations:
            if not isinstance(alloc, mybir.MemoryLocationSet):
                continue
            name = alloc.memorylocations[0].name
            if alloc.kind == "ExternalInput":
                if name != partition_name and name != (
                        nc.dbg_addr.name if nc.dbg_addr else None):
                    in_names.append(name)
            elif alloc.kind == "ExternalOutput":
                out_names.append(name)
                shape = tuple(alloc.tensor_shape)
                dtype = mybir.dt.np(alloc.dtype)
                out_avals.append(jax.core.ShapedArray(shape, dtype))
                zero_outs.append(np.zeros(shape, dtype))
        self.in_names, self.out_names = in_names, out_names
        self.out_avals, self.zero_outs = out_avals, zero_outs
        n_params = len(in_names)
        all_in_names = list(in_names) + list(out_names)
        has_dbg = nc.dbg_addr is not None
        if has_dbg:
            all_in_names.append(nc.dbg_addr.name)
        if partition_name is not None:
            all_in_names.append(partition_name)

        def _body(*args):
            operands = list(args)
            if has_dbg:
                operands.append(jax.numpy.zeros((1, 2), jax.numpy.uint32))
            if partition_name is not None:
                operands.append(partition_id_tensor())
            outs = _bass_exec_p.bind(
                *operands, out_avals=tuple(out_avals),
                in_names=tuple(all_in_names), out_names=tuple(out_names),
                lowering_input_output_aliases=(),
                sim_require_finite=False, sim_require_nnan=False, nc=nc)
            return tuple(outs)

        devices = jax.devices()[:n_cores]
        assert len(devices) == n_cores
        mesh = Mesh(np.asarray(devices), ("core",))
        in_specs = (PartitionSpec("core"),) * (n_params + len(out_names))
        out_specs = (PartitionSpec("core"),) * len(out_names)
        self.fn = jax.jit(
            shard_map(_body, mesh=mesh, in_specs=in_specs,
                      out_specs=out_specs, check_rep=False),
            keep_unused=True)

    def prepare(self, in_maps):
        per_core = [[np.ascontiguousarray(m[nm]) for nm in self.in_names]
                    for m in in_maps]
        concat_in = [
            np.concatenate([per_core[c][i] for c in range(self.n_cores)],
                           axis=0)
            for i in range(len(self.in_names))]
        concat_zero = [
            np.zeros((self.n_cores * z.shape[0], *z.shape[1:]), z.dtype)
            for z in self.zero_outs]
        args = [self.jax.device_put(a) for a in concat_in + concat_zero]
        for a in args:
            a.block_until_ready()
        return args

    def run(self, args):
        outs = self.fn(*args)
        self.jax.block_until_ready(outs)
        return outs

    def results(self, outs):
        res = []
        for c in range(self.n_cores):
            m = {}
            for i, nm in enumerate(self.out_names):
                m[nm] = np.asarray(outs[i]).reshape(
                    self.n_cores, *self.out_avals[i].shape)[c]
            res.append(m)
        return res


# -------------------------------------------------------------------- kernel

_CACHE = {}

N_FULL, E_FULL, G_FULL, NCORES = 50000, 800000, 512, 8


def kernel(x, edge_index, batch,
           W0, a_src0, a_dst0, bias0,
           W1, a_src1, a_dst1, bias1,
           W2, a_src2, a_dst2, bias2,
           lin_w, lin_b):
    x = np.asarray(x, np.float32)
    edge_index = np.asarray(edge_index, np.int64)
    batch = np.asarray(batch, np.int64)
    N, E, G = x.shape[0], edge_index.shape[1], G_FULL

    cfg, in_maps = host_prep(
        x, edge_index, batch,
        [np.asarray(W0, np.float32), np.asarray(W1, np.float32),
         np.asarray(W2, np.float32)],
        [np.asarray(a_src0, np.float32), np.asarray(a_src1, np.float32),
         np.asarray(a_src2, np.float32)],
        [np.asarray(a_dst0, np.float32), np.asarray(a_dst1, np.float32),
         np.asarray(a_dst2, np.float32)],
        [np.asarray(bias0, np.float32), np.asarray(bias1, np.float32),
         np.asarray(bias2, np.float32)],
        np.asarray(lin_w, np.float32), np.asarray(lin_b, np.float32),
        N, E, G, NCORES)

    key = (cfg["NL"], cfg["NT"], cfg["GP"], cfg["m_lo"], cfg["m_hi"],
           cfg["SM"], cfg["ncores"])
    if key not in _CACHE:
        nc = build_program(cfg)
        _CACHE[key] = (nc, SpmdRunner(nc, NCORES))
    nc, runner = _CACHE[key]

    args = runner.prepare(in_maps)
    outs = runner.run(args)
    res = runner.results(outs)
    return res[0]["y"][:G].astype(np.float32)
